# revision 27
# baseline (speedup 1.0000x reference)
"""Trainium2 Bass kernel for causal multi-head attention.

Problem: B=2, C=2048, H=1024, 16 heads, head_dim=64, float32.
    qkv = x @ Wqkv.T + b ; causal softmax attention ; out = att @ Wo.T + b

Sharding over 8 NeuronCores: core i owns heads {2i, 2i+1} for BOTH batches
(tensor parallel over heads). After attention, two AllToAlls (one per
q-column half) redistribute per-head attention outputs; each core applies
the output projection for rows (batch i//4, query quarter i%4). Host
concatenates the 8 [512, 1024] slices.

Structure notes:
  - xT [1024, 4096] = concat(x[0].T, x[1].T)     (host pre-transposed)
  - q^T/k^T in [2 heads x 64d, t] float32r (full-rate PE); v PE-transposed
    into [t, d] bf16 tiles with a ones column so the attention matmul also
    emits the softmax row sums.
  - batch-1 QKV is interleaved with batch-0 attention so ScalarE (the
    attention bottleneck: one exp per [128 k, 2x512 q] tile) starts ~40us
    earlier and the PE never drains.
  - S^T tiles [128 k, 1024]: both heads via PE row tiling, one fused exp
    (scale=1/8, no max subtraction: logits ~N(0,1)); causal masking via
    0/1 bf16 mask multiply on diagonal tiles; AV software-pipelined one
    k-block behind S^T/exp.
  - A2A payload bf16; a keep-warm matmul chain (dependency-pinned to the
    end of attention) holds the PE clock up through the collective.
"""
import math
import numpy as np

C, H, NH, HD = 2048, 1024, 16, 64
B = 2
NCORES = 8
KEEPWARM = 64

_cache = {}


def _build():
    import concourse.bass as bass
    import concourse.bacc as bacc
    import concourse.tile as tile
    import concourse.mybir as mybir

    dt = mybir.dt
    f32 = dt.float32
    f32r = dt.float32r
    bf16 = dt.bfloat16
    AF = mybir.ActivationFunctionType

    nc = bacc.Bacc("TRN2", target_bir_lowering=False, debug=False,
                   enable_asserts=True, num_devices=NCORES)

    def din(name, shape, d=f32):
        return nc.dram_tensor(name, shape, d, kind="ExternalInput").ap()

    xT = din("xT", [1024, 4096], bf16)
    wqkT = din("wqkT", [1024, 256], bf16)
    qk_bias = din("qk_bias", [128, 2])
    wvT = din("wvT", [1024, 128], bf16)
    vb_bcast = din("vb_bcast", [128, 128])
    masks = din("masks", [128, 2048], bf16)
    ident = din("ident", [128, 128], bf16)
    ones64 = din("ones64", [128, 64], f32r)
    onesbf = din("onesbf", [128, 64], bf16)
    woT = din("woT", [1024, 1024], bf16)
    wob_bcast = din("wob_bcast", [128, 1024])
    y_out = nc.dram_tensor("y", [512, 1024], f32, kind="ExternalOutput").ap()

    with tile.TileContext(nc) as tc:
        ctx_lp = nc.allow_low_precision(
            reason="float32r/bf16 operands; all matmuls accumulate in f32 PSUM")
        ctx_lp.__enter__()
        with (
            tc.tile_pool(name="const", bufs=1) as const_pool,
            tc.tile_pool(name="persist", bufs=1) as persist,
            tc.tile_pool(name="dram", bufs=1, space="DRAM") as dram,
            tc.tile_pool(name="qkvps", bufs=2, space="PSUM") as qkv_ps,
            tc.tile_pool(name="sps", bufs=2, space="PSUM") as s_ps,
            tc.tile_pool(name="avps", bufs=1, space="PSUM") as av_ps,
            tc.tile_pool(name="xt", bufs=4) as xt_pool,
            tc.tile_pool(name="psb", bufs=8) as p_pool,
            tc.tile_pool(name="attsb", bufs=4) as att_pool,
            tc.tile_pool(name="recsb", bufs=4) as rec_pool,
        ):
            # -------- weights + first inputs (emitted first: on critical path)
            wqk_sb = const_pool.tile([128, 8 * 256], bf16, tag="wqk")  # hb-major
            wv_sb = const_pool.tile([128, 8 * 128], bf16, tag="wv")
            for hb in range(8):
                nc.sync.dma_start(wqk_sb[:, 256 * hb:256 * hb + 256],
                                  wqkT[128 * hb:128 * hb + 128, :])
                nc.sync.dma_start(wv_sb[:, 128 * hb:128 * hb + 128],
                                  wvT[128 * hb:128 * hb + 128, :])
            qkb_sb = const_pool.tile([128, 2], f32, tag="qkb")
            nc.sync.dma_start(qkb_sb[:], qk_bias)
            vbb_sb = const_pool.tile([128, 128], f32, tag="vbb")
            nc.sync.dma_start(vbb_sb[:], vb_bcast)
            ident_sb = const_pool.tile([128, 128], bf16, tag="ident")
            nc.sync.dma_start(ident_sb[:], ident)

            # -------- persistent activations
            qT_sb = persist.tile([128, 4096], bf16, tag="qT")
            kT_sb = persist.tile([128, 4096], bf16, tag="kT")
            vT_sb = persist.tile([128, 4096], bf16, tag="vT")
            v_sb = persist.tile([128, 32 * 130], bf16, tag="v")   # [t, d|1] slots

            # -------- constants needed later (gpsimd queue, off critical path)
            masks_sb = const_pool.tile([128, 2048], bf16, tag="masks")
            nc.gpsimd.dma_start(masks_sb[:], masks)
            ones64_sb = const_pool.tile([128, 64], f32r, tag="ones64")
            nc.gpsimd.dma_start(ones64_sb[:], ones64)
            onesbf_sb = const_pool.tile([128, 64], bf16, tag="onesbf")
            nc.gpsimd.dma_start(onesbf_sb[:], onesbf)
            wob_sb = const_pool.tile([128, 1024], f32, tag="wob")
            nc.gpsimd.dma_start(wob_sb[:], wob_bcast)
            wo_sb = const_pool.tile([128, 8 * 1024], bf16, tag="wot")
            for cb in range(8):
                nc.gpsimd.dma_start(wo_sb[:, 1024 * cb:1024 * cb + 1024],
                                    woT[128 * cb:128 * cb + 128, :])

            # ones columns of v_sb: cols {130*s + 64, 130*s + 129}
            ones_view = v_sb[:].rearrange("p (s h e) -> p s h e", s=32, h=2, e=65)
            nc.vector.tensor_copy(
                ones_view[:, :, :, 64],
                onesbf_sb[:].rearrange("p (s h) -> p s h", s=32, h=2))

            a2a_in0 = dram.tile([1024, 256], bf16, tag="a2a_in0")
            a2a_out0 = dram.tile([1024, 256], bf16, tag="a2a_out0")
            a2a_in1 = dram.tile([1024, 256], bf16, tag="a2a_in1")
            a2a_out1 = dram.tile([1024, 256], bf16, tag="a2a_out1")

            # ---------------- building blocks ----------------
            def qkv_ttile(tt):
                """QKV projection for one 512-wide t tile."""
                xt = xt_pool.tile([128, 8 * 512], bf16, tag="xt", name="xt")
                for hb in range(8):
                    nc.sync.dma_start(
                        xt[:, 512 * hb:512 * hb + 512],
                        xT[128 * hb:128 * hb + 128, 512 * tt:512 * tt + 512])
                for ob in range(2):
                    ps = qkv_ps.tile([128, 512], f32, tag="qkv", name="qkvp")
                    for hb in range(8):
                        nc.tensor.matmul(
                            ps[:],
                            wqk_sb[:, 256 * hb + 128 * ob:256 * hb + 128 * ob + 128],
                            xt[:, 512 * hb:512 * hb + 512],
                            start=(hb == 0), stop=(hb == 7))
                    dest = qT_sb if ob == 0 else kT_sb
                    nc.vector.tensor_scalar_add(
                        dest[:, 512 * tt:512 * tt + 512], ps[:],
                        qkb_sb[:, ob:ob + 1])
                ps = qkv_ps.tile([128, 512], f32, tag="qkv", name="qkvp2")
                for hb in range(8):
                    nc.tensor.matmul(
                        ps[:],
                        wv_sb[:, 128 * hb:128 * hb + 128],
                        xt[:, 512 * hb:512 * hb + 512],
                        start=(hb == 0), stop=(hb == 7))
                nc.vector.tensor_copy(vT_sb[:, 512 * tt:512 * tt + 512], ps[:])

            def v_transpose(b, tbs=range(16)):
                """v^T -> v tiles [128 t, 128 d] + bias, into bf16 v_sb."""
                for tb in tbs:
                    slot = 16 * b + tb
                    ps = qkv_ps.tile([128, 128], bf16, tag="qkv", name="vt")
                    nc.tensor.transpose(
                        ps[:], vT_sb[:, 2048 * b + 128 * tb:2048 * b + 128 * tb + 128],
                        ident_sb[:])
                    dv = v_sb[:].rearrange("p (s h e) -> p s h e", s=32, h=2, e=65)
                    sv = ps[:].rearrange("p (h e) -> p h e", h=2, e=64)
                    bv = vbb_sb[:].rearrange("p (h e) -> p h e", h=2, e=64)
                    nc.vector.tensor_add(dv[:, slot, :, 0:64], sv, bv)

            last_att = [None]

            def attention_qtile(b, qt):
                nkb = 4 * (qt + 1)
                avA = av_ps.tile([65, 512], f32, tag="avA", name="avA")
                avB = av_ps.tile([65, 512], f32, tag="avB", name="avB")
                qlo = 2048 * b + 512 * qt
                pend = None   # software pipeline: AV runs one kb behind S^T/exp
                for kb in range(nkb):
                    klo = 2048 * b + 128 * kb
                    roff = kb - 4 * qt
                    # columns q < 128*roff of a diagonal tile are fully masked:
                    # skip them in the S^T and AV streams (stale PSUM there is
                    # exp'd but never consumed)
                    lo = 128 * roff if roff > 0 else 0
                    sAB = s_ps.tile([128, 1024], f32, tag="s", name="sAB")
                    nc.tensor.matmul(
                        sAB[:, lo:512], kT_sb[0:64, klo:klo + 128],
                        qT_sb[0:64, qlo + lo:qlo + 512])
                    nc.tensor.matmul(
                        sAB[:, 512 + lo:1024], kT_sb[64:128, klo:klo + 128],
                        qT_sb[64:128, qlo + lo:qlo + 512])
                    pAB = p_pool.tile([128, 1024], bf16, tag="p", name="pAB")
                    nc.scalar.activation(pAB[:], sAB[:], AF.Exp,
                                         scale=1.0 / math.sqrt(HD))
                    if roff >= 0:
                        m = masks_sb[:, 512 * roff + lo:512 * roff + 512]
                        nc.vector.tensor_mul(pAB[:, lo:512], pAB[:, lo:512], m)
                        nc.vector.tensor_mul(pAB[:, 512 + lo:1024],
                                             pAB[:, 512 + lo:1024], m)
                    if pend is not None:
                        pkb, ppAB, plo = pend
                        pslot = 16 * b + pkb
                        nc.tensor.matmul(
                            avA[:, plo:512], v_sb[:, 130 * pslot:130 * pslot + 65],
                            ppAB[:, plo:512], start=(pkb == 0), stop=False)
                        nc.tensor.matmul(
                            avB[:, plo:512], v_sb[:, 130 * pslot + 65:130 * pslot + 130],
                            ppAB[:, 512 + plo:1024], start=(pkb == 0), stop=False)
                    pend = (kb, pAB, lo)
                pkb, ppAB, plo = pend
                pslot = 16 * b + pkb
                nc.tensor.matmul(
                    avA[:, plo:512], v_sb[:, 130 * pslot:130 * pslot + 65],
                    ppAB[:, plo:512], start=(pkb == 0), stop=True)
                nc.tensor.matmul(
                    avB[:, plo:512], v_sb[:, 130 * pslot + 65:130 * pslot + 130],
                    ppAB[:, 512 + plo:1024], start=(pkb == 0), stop=True)
                # normalize + ship halves to the two a2a buffers
                j = 4 * b + qt
                for h, av in ((0, avA), (1, avB)):
                    lrow = rec_pool.tile([128, 512], f32r, tag="lrow", name="lrow")
                    nc.vector.tensor_copy(lrow[64:65, :], av[64:65, :])
                    rcp = s_ps.tile([64, 512], f32, tag="s", name="rcp")
                    nc.tensor.matmul(rcp[:], ones64_sb[64:65, :], lrow[64:65, :])
                    rcp_sb = rec_pool.tile([64, 512], f32, tag="recsb",
                                           name="rcp_sb")
                    nc.vector.reciprocal_approx_fast(rcp_sb[:], rcp[:])
                    att = att_pool.tile([64, 512], bf16, tag="att", name="att")
                    nc.vector.tensor_mul(att[:], av[0:64, :], rcp_sb[:])
                    last_att[0] = att
                    dst = 128 * j + 64 * h
                    nc.gpsimd.dma_start(a2a_in0[dst:dst + 64, :], att[:, 0:256])
                    nc.gpsimd.dma_start(a2a_in1[dst:dst + 64, :], att[:, 256:512])

            # ---------------- emission: interleaved schedule ----------------
            # attention(0, qt) needs q^T/k^T t-tiles <= qt and v slots
            # <= 4*qt+4, so it can start right after t-tile qt of batch 0.
            for qt in range(4):
                qkv_ttile(qt)
                v_transpose(0, range(4 * qt, 4 * qt + 4))
                attention_qtile(0, qt)
                qkv_ttile(4 + qt)
            v_transpose(1)
            for qt in range(4):
                attention_qtile(1, qt)

            # ---------------- A2A + output projection ----------------
            nc.gpsimd.collective_compute(
                "AllToAll", mybir.AluOpType.bypass,
                replica_groups=[list(range(NCORES))],
                ins=[a2a_in0[:].opt()], outs=[a2a_out0[:].opt()])
            nc.gpsimd.collective_compute(
                "AllToAll", mybir.AluOpType.bypass,
                replica_groups=[list(range(NCORES))],
                ins=[a2a_in1[:].opt()], outs=[a2a_out1[:].opt()])

            with (
                tc.tile_pool(name="attin", bufs=1) as att_in_pool,
                tc.tile_pool(name="ysb", bufs=4) as y_pool,
            ):
                # keep-warm chain pinned behind the end of attention via a
                # data dependency on the last att tile
                kw_src = att_in_pool.tile([64, 128], bf16, tag="kwsrc",
                                          name="kw_src")
                nc.vector.tensor_copy(kw_src[:], last_att[0][:, 0:128])
                junk = qkv_ps.tile([128, 512], f32, tag="qkv", name="junk")
                for _ in range(KEEPWARM):
                    nc.tensor.matmul(junk[:], kw_src[:], qT_sb[0:64, 0:512],
                                     start=True, stop=True)

                att_sbs = []
                for half, a2a_out_h in ((0, a2a_out0), (1, a2a_out1)):
                    att_sb = att_in_pool.tile([128, 8 * 256], bf16,
                                              tag=f"attfull{half}",
                                              name=f"att_sb{half}")
                    att_sbs.append(att_sb)
                    for cb in range(8):
                        nc.sync.dma_start(att_sb[:, 256 * cb:256 * cb + 256],
                                          a2a_out_h[128 * cb:128 * cb + 128, :])
                for half in range(2):
                    if half == 1:
                        # second keep-warm chain: fills the remainder of the
                        # A2A_1 window after Wo-half0's matmuls so Wo-half1
                        # starts at full PE clock (f32 junk = long matmuls)
                        junk2 = qkv_ps.tile([128, 512], f32, tag="qkv",
                                            name="junk2")
                        for _ in range(14):
                            nc.tensor.matmul(junk2[:], ysb[0:64, 0:128],
                                             wob_sb[0:64, 0:512],
                                             start=True, stop=True)
                    att_sb = att_sbs[half]
                    for qh in range(2):      # local q block within the half
                        qb = 2 * half + qh
                        for jt in range(2):
                            ps = qkv_ps.tile([128, 512], f32, tag="qkv", name="yps")
                            for cb in range(8):
                                nc.tensor.matmul(
                                    ps[:],
                                    att_sb[:, 256 * cb + 128 * qh:256 * cb + 128 * qh + 128],
                                    wo_sb[:, 1024 * cb + 512 * jt:1024 * cb + 512 * jt + 512],
                                    start=(cb == 0), stop=(cb == 7))
                            ysb = y_pool.tile([128, 512], f32, tag="ysb",
                                              name="ysb")
                            nc.vector.tensor_add(ysb[:], ps[:],
                                                 wob_sb[:, 512 * jt:512 * jt + 512])
                            nc.sync.dma_start(
                                y_out[128 * qb:128 * qb + 128, 512 * jt:512 * jt + 512],
                                ysb[:])
        ctx_lp.__exit__(None, None, None)

    nc.compile()
    return nc


def host_prep(x, Wqkv_w, Wqkv_b, Wo_w, Wo_b):
    import ml_dtypes
    bf16 = ml_dtypes.bfloat16

    x = np.asarray(x, np.float32)
    Wqkv_w = np.asarray(Wqkv_w, np.float32)
    Wqkv_b = np.asarray(Wqkv_b, np.float32)
    Wo_w = np.asarray(Wo_w, np.float32)
    Wo_b = np.asarray(Wo_b, np.float32)

    xT = np.ascontiguousarray(
        np.concatenate([x[0].T, x[1].T], axis=1), dtype=np.float32).astype(bf16)
    masks = np.zeros((128, 2048), np.float32)
    for roff in range(4):
        kk = np.arange(128)[:, None] + roff * 128
        qq = np.arange(512)[None, :]
        masks[:, 512 * roff:512 * roff + 512] = (kk <= qq).astype(np.float32)
    masks = masks.astype(bf16)
    ident = np.eye(128, dtype=np.float32).astype(bf16)
    ones64 = np.ones((128, 64), np.float32)
    onesbf = np.ones((128, 64), bf16)
    woT = np.ascontiguousarray(Wo_w.T).astype(bf16)
    wob_bcast = np.tile(Wo_b[None, :], (128, 1)).astype(np.float32)

    in_maps = []
    for i in range(NCORES):
        hA, hB = 2 * i, 2 * i + 1
        rows_qk = np.r_[64 * hA:64 * hA + 64, 64 * hB:64 * hB + 64,
                        1024 + 64 * hA:1024 + 64 * hA + 64,
                        1024 + 64 * hB:1024 + 64 * hB + 64]
        wqkT = np.ascontiguousarray(Wqkv_w[rows_qk].T).astype(bf16)
        qkb = np.ascontiguousarray(Wqkv_b[rows_qk].reshape(2, 128).T)
        rows_v = np.r_[2048 + 64 * hA:2048 + 64 * hA + 64,
                       2048 + 64 * hB:2048 + 64 * hB + 64]
        wvT = np.ascontiguousarray(Wqkv_w[rows_v].T).astype(bf16)
        vbb = np.tile(Wqkv_b[rows_v][None, :], (128, 1)).astype(np.float32)
        in_maps.append(dict(
            xT=xT, wqkT=wqkT, qk_bias=qkb, wvT=wvT, vb_bcast=vbb,
            masks=masks, ident=ident, ones64=ones64, onesbf=onesbf, woT=woT,
            wob_bcast=wob_bcast))
    return in_maps


def _ensure_ntff_hook_module():
    """run_bass_kernel_spmd(trace=True) under axon imports
    antenv.axon_hooks; provide a working ctypes-based fallback if the
    environment doesn't ship one so tracing (e.g. via BASS_TRACE=1) works."""
    import importlib
    import sys
    import types
    try:
        importlib.import_module("antenv.axon_hooks")
        return
    except ImportError:
        pass
    import contextlib
    import ctypes

    mod = types.ModuleType("antenv.axon_hooks")
    state = {"hook": None}

    def set_axon_ntff_profile_hook(h):
        state["hook"] = h

    def _make():
        try:
            lib = ctypes.CDLL("/opt/axon/libaxon_pjrt.so")
        except OSError:
            return None
        if not hasattr(lib, "axon_start_nrt_profile"):
            return None
        lib.axon_start_nrt_profile.argtypes = [
            ctypes.POINTER(ctypes.c_int64), ctypes.c_size_t]
        lib.axon_start_nrt_profile.restype = ctypes.c_int64
        lib.axon_stop_nrt_profile.argtypes = [ctypes.c_char_p]
        lib.axon_stop_nrt_profile.restype = ctypes.c_int64

        @contextlib.contextmanager
        def _hook(output_dir, device_ids):
            import jax
            jax.devices()
            if device_ids:
                ids = (ctypes.c_int64 * len(device_ids))(*device_ids)
                rc = lib.axon_start_nrt_profile(ids, len(device_ids))
            else:
                rc = lib.axon_start_nrt_profile(None, 0)
            if rc != 0:
                raise RuntimeError(f"axon_start_nrt_profile rc={rc}")
            try:
                yield
            finally:
                lib.axon_stop_nrt_profile(str(output_dir).encode())

        return _hook

    def get_axon_ntff_profile_hook():
        if state["hook"] is None:
            state["hook"] = _make()
        return state["hook"]

    mod.set_axon_ntff_profile_hook = set_axon_ntff_profile_hook
    mod.get_axon_ntff_profile_hook = get_axon_ntff_profile_hook
    try:
        import antenv
        sys.modules["antenv.axon_hooks"] = mod
        antenv.axon_hooks = mod
    except ImportError:
        pass


def kernel(x, Wqkv_w, Wqkv_b, Wo_w, Wo_b):
    from concourse import bass_utils

    _ensure_ntff_hook_module()

    if "nc" not in _cache:
        _cache["nc"] = _build()
    nc = _cache["nc"]

    in_maps = host_prep(x, Wqkv_w, Wqkv_b, Wo_w, Wo_b)
    res = bass_utils.run_bass_kernel_spmd(nc, in_maps, core_ids=list(range(NCORES)))
    _cache["last_results"] = res

    out = np.zeros((B, C, H), np.float32)
    for i in range(NCORES):
        out[i // 4, 512 * (i % 4):512 * (i % 4) + 512, :] = res.results[i]["y"]
    return out


# revision 29
# speedup vs baseline: 1.0729x; 1.0729x over previous
"""Trainium2 Bass kernel for causal multi-head attention.

Problem: B=2, C=2048, H=1024, 16 heads, head_dim=64, float32.
    qkv = x @ Wqkv.T + b ; causal softmax attention ; out = att @ Wo.T + b

Sharding over 8 NeuronCores: core i owns heads {2i, 2i+1} for BOTH batches
(tensor parallel over heads). After attention, two AllToAlls (one per
q-column half) redistribute per-head attention outputs; each core applies
the output projection for rows (batch i//4, query quarter i%4). Host
concatenates the 8 [512, 1024] slices.

Structure notes:
  - xT [1024, 4096] = concat(x[0].T, x[1].T)     (host pre-transposed)
  - q^T/k^T in [2 heads x 64d, t] float32r (full-rate PE); v PE-transposed
    into [t, d] bf16 tiles with a ones column so the attention matmul also
    emits the softmax row sums.
  - batch-1 QKV is interleaved with batch-0 attention so ScalarE (the
    attention bottleneck: one exp per [128 k, 2x512 q] tile) starts ~40us
    earlier and the PE never drains.
  - S^T tiles [128 k, 1024]: both heads via PE row tiling, one fused exp
    (scale=1/8, no max subtraction: logits ~N(0,1)); causal masking via
    0/1 bf16 mask multiply on diagonal tiles; AV software-pipelined one
    k-block behind S^T/exp.
  - A2A payload bf16; a keep-warm matmul chain (dependency-pinned to the
    end of attention) holds the PE clock up through the collective.
"""
import math
import numpy as np

C, H, NH, HD = 2048, 1024, 16, 64
B = 2
NCORES = 8
KEEPWARM = 52

_cache = {}


def _build():
    import concourse.bass as bass
    import concourse.bacc as bacc
    import concourse.tile as tile
    import concourse.mybir as mybir

    dt = mybir.dt
    f32 = dt.float32
    f32r = dt.float32r
    bf16 = dt.bfloat16
    AF = mybir.ActivationFunctionType

    nc = bacc.Bacc("TRN2", target_bir_lowering=False, debug=False,
                   enable_asserts=True, num_devices=NCORES)

    def din(name, shape, d=f32):
        return nc.dram_tensor(name, shape, d, kind="ExternalInput").ap()

    xT = din("xT", [1024, 4096], bf16)
    wqkT = din("wqkT", [1024, 256], bf16)
    qk_bias = din("qk_bias", [128, 2])
    wvT = din("wvT", [1024, 128], bf16)
    vb_bcast = din("vb_bcast", [128, 128])
    masks = din("masks", [128, 2048], bf16)
    ident = din("ident", [128, 128], bf16)
    ones64 = din("ones64", [128, 64], f32r)
    onesbf = din("onesbf", [128, 64], bf16)
    woT = din("woT", [1024, 1024], bf16)
    wob_bcast = din("wob_bcast", [128, 1024])
    y_out = nc.dram_tensor("y", [512, 1024], f32, kind="ExternalOutput").ap()

    with tile.TileContext(nc) as tc:
        ctx_lp = nc.allow_low_precision(
            reason="float32r/bf16 operands; all matmuls accumulate in f32 PSUM")
        ctx_lp.__enter__()
        with (
            tc.tile_pool(name="const", bufs=1) as const_pool,
            tc.tile_pool(name="persist", bufs=1) as persist,
            tc.tile_pool(name="dram", bufs=1, space="DRAM") as dram,
            tc.tile_pool(name="qkvps", bufs=2, space="PSUM") as qkv_ps,
            tc.tile_pool(name="sps", bufs=2, space="PSUM") as s_ps,
            tc.tile_pool(name="avps", bufs=1, space="PSUM") as av_ps,
            tc.tile_pool(name="xt", bufs=3) as xt_pool,
            tc.tile_pool(name="psb", bufs=10) as p_pool,
            tc.tile_pool(name="attsb", bufs=6) as att_pool,
            tc.tile_pool(name="recsb", bufs=6) as rec_pool,
        ):
            # -------- weights + first inputs (emitted first: on critical path)
            wqk_sb = const_pool.tile([128, 8 * 256], bf16, tag="wqk")  # hb-major
            wv_sb = const_pool.tile([128, 8 * 128], bf16, tag="wv")
            for hb in range(8):
                nc.sync.dma_start(wqk_sb[:, 256 * hb:256 * hb + 256],
                                  wqkT[128 * hb:128 * hb + 128, :])
                nc.sync.dma_start(wv_sb[:, 128 * hb:128 * hb + 128],
                                  wvT[128 * hb:128 * hb + 128, :])
            qkb_sb = const_pool.tile([128, 2], f32, tag="qkb")
            nc.gpsimd.dma_start(qkb_sb[:], qk_bias)
            vbb_sb = const_pool.tile([128, 128], f32, tag="vbb")
            nc.gpsimd.dma_start(vbb_sb[:], vb_bcast)
            ident_sb = const_pool.tile([128, 128], bf16, tag="ident")
            nc.gpsimd.dma_start(ident_sb[:], ident)

            # -------- persistent activations
            qT_sb = persist.tile([128, 4096], bf16, tag="qT")
            kT_sb = persist.tile([128, 4096], bf16, tag="kT")
            vT_sb = persist.tile([128, 4096], bf16, tag="vT")
            v_sb = persist.tile([128, 32 * 130], bf16, tag="v")   # [t, d|1] slots

            # -------- constants needed later (gpsimd queue, off critical path)
            masks_sb = const_pool.tile([128, 2048], bf16, tag="masks")
            nc.gpsimd.dma_start(masks_sb[:], masks)
            ones64_sb = const_pool.tile([128, 64], f32r, tag="ones64")
            nc.gpsimd.dma_start(ones64_sb[:], ones64)
            onesbf_sb = const_pool.tile([128, 64], bf16, tag="onesbf")
            nc.gpsimd.dma_start(onesbf_sb[:], onesbf)
            wob_sb = const_pool.tile([128, 1024], f32, tag="wob")
            nc.gpsimd.dma_start(wob_sb[:], wob_bcast)
            wo_sb = const_pool.tile([128, 8 * 1024], bf16, tag="wot")
            for cb in range(8):
                nc.gpsimd.dma_start(wo_sb[:, 1024 * cb:1024 * cb + 1024],
                                    woT[128 * cb:128 * cb + 128, :])

            # ones columns of v_sb: cols {130*s + 64, 130*s + 129}
            ones_view = v_sb[:].rearrange("p (s h e) -> p s h e", s=32, h=2, e=65)
            nc.vector.tensor_copy(
                ones_view[:, :, :, 64],
                onesbf_sb[:].rearrange("p (s h) -> p s h", s=32, h=2))

            a2a_in0 = dram.tile([1024, 256], bf16, tag="a2a_in0")
            a2a_out0 = dram.tile([1024, 256], bf16, tag="a2a_out0")
            a2a_in1 = dram.tile([1024, 256], bf16, tag="a2a_in1")
            a2a_out1 = dram.tile([1024, 256], bf16, tag="a2a_out1")

            # ---------------- building blocks ----------------
            def qkv_ttile(tt):
                """QKV projection for one 512-wide t tile."""
                xt = xt_pool.tile([128, 8 * 512], bf16, tag="xt", name="xt")
                for hb in range(8):
                    nc.sync.dma_start(
                        xt[:, 512 * hb:512 * hb + 512],
                        xT[128 * hb:128 * hb + 128, 512 * tt:512 * tt + 512])
                for ob in range(2):
                    ps = qkv_ps.tile([128, 512], f32, tag="qkv", name="qkvp")
                    for hb in range(8):
                        nc.tensor.matmul(
                            ps[:],
                            wqk_sb[:, 256 * hb + 128 * ob:256 * hb + 128 * ob + 128],
                            xt[:, 512 * hb:512 * hb + 512],
                            start=(hb == 0), stop=(hb == 7))
                    dest = qT_sb if ob == 0 else kT_sb
                    nc.vector.tensor_scalar_add(
                        dest[:, 512 * tt:512 * tt + 512], ps[:],
                        qkb_sb[:, ob:ob + 1])
                ps = qkv_ps.tile([128, 512], f32, tag="qkv", name="qkvp2")
                for hb in range(8):
                    nc.tensor.matmul(
                        ps[:],
                        wv_sb[:, 128 * hb:128 * hb + 128],
                        xt[:, 512 * hb:512 * hb + 512],
                        start=(hb == 0), stop=(hb == 7))
                nc.vector.tensor_copy(vT_sb[:, 512 * tt:512 * tt + 512], ps[:])

            def v_transpose(b, tbs=range(16)):
                """v^T -> v tiles [128 t, 128 d] + bias, into bf16 v_sb."""
                for tb in tbs:
                    slot = 16 * b + tb
                    ps = qkv_ps.tile([128, 128], bf16, tag="qkv", name="vt")
                    nc.tensor.transpose(
                        ps[:], vT_sb[:, 2048 * b + 128 * tb:2048 * b + 128 * tb + 128],
                        ident_sb[:])
                    dv = v_sb[:].rearrange("p (s h e) -> p s h e", s=32, h=2, e=65)
                    sv = ps[:].rearrange("p (h e) -> p h e", h=2, e=64)
                    bv = vbb_sb[:].rearrange("p (h e) -> p h e", h=2, e=64)
                    nc.vector.tensor_add(dv[:, slot, :, 0:64], sv, bv)

            last_att = [None]

            def attention_qtile(b, qt):
                nkb = 4 * (qt + 1)
                avA = av_ps.tile([65, 512], f32, tag="avA", name="avA")
                avB = av_ps.tile([65, 512], f32, tag="avB", name="avB")
                qlo = 2048 * b + 512 * qt
                pend = None   # software pipeline: AV runs one kb behind S^T/exp
                for kb in range(nkb):
                    klo = 2048 * b + 128 * kb
                    roff = kb - 4 * qt
                    # columns q < 128*roff of a diagonal tile are fully masked:
                    # skip them in the S^T and AV streams (stale PSUM there is
                    # exp'd but never consumed)
                    lo = 128 * roff if roff > 0 else 0
                    sAB = s_ps.tile([128, 1024], f32, tag="s", name="sAB")
                    nc.tensor.matmul(
                        sAB[:, lo:512], kT_sb[0:64, klo:klo + 128],
                        qT_sb[0:64, qlo + lo:qlo + 512])
                    nc.tensor.matmul(
                        sAB[:, 512 + lo:1024], kT_sb[64:128, klo:klo + 128],
                        qT_sb[64:128, qlo + lo:qlo + 512])
                    pAB = p_pool.tile([128, 1024], bf16, tag="p", name="pAB")
                    nc.scalar.activation(pAB[:], sAB[:], AF.Exp,
                                         scale=1.0 / math.sqrt(HD))
                    if roff >= 0:
                        m = masks_sb[:, 512 * roff + lo:512 * roff + 512]
                        nc.vector.tensor_mul(pAB[:, lo:512], pAB[:, lo:512], m)
                        nc.vector.tensor_mul(pAB[:, 512 + lo:1024],
                                             pAB[:, 512 + lo:1024], m)
                    if pend is not None:
                        pkb, ppAB, plo = pend
                        pslot = 16 * b + pkb
                        nc.tensor.matmul(
                            avA[:, plo:512], v_sb[:, 130 * pslot:130 * pslot + 65],
                            ppAB[:, plo:512], start=(pkb == 0), stop=False)
                        nc.tensor.matmul(
                            avB[:, plo:512], v_sb[:, 130 * pslot + 65:130 * pslot + 130],
                            ppAB[:, 512 + plo:1024], start=(pkb == 0), stop=False)
                    pend = (kb, pAB, lo)
                pkb, ppAB, plo = pend
                pslot = 16 * b + pkb
                nc.tensor.matmul(
                    avA[:, plo:512], v_sb[:, 130 * pslot:130 * pslot + 65],
                    ppAB[:, plo:512], start=(pkb == 0), stop=True)
                nc.tensor.matmul(
                    avB[:, plo:512], v_sb[:, 130 * pslot + 65:130 * pslot + 130],
                    ppAB[:, 512 + plo:1024], start=(pkb == 0), stop=True)
                # normalize + ship halves to the two a2a buffers
                j = 4 * b + qt
                for h, av in ((0, avA), (1, avB)):
                    lrow = rec_pool.tile([128, 512], f32r, tag="lrow", name="lrow")
                    nc.vector.tensor_copy(lrow[64:65, :], av[64:65, :])
                    rcp = s_ps.tile([64, 512], f32, tag="s", name="rcp")
                    nc.tensor.matmul(rcp[:], ones64_sb[64:65, :], lrow[64:65, :])
                    rcp_sb = rec_pool.tile([64, 512], f32, tag="recsb",
                                           name="rcp_sb")
                    nc.vector.reciprocal_approx_fast(rcp_sb[:], rcp[:])
                    att = att_pool.tile([64, 512], bf16, tag="att", name="att")
                    nc.vector.tensor_mul(att[:], av[0:64, :], rcp_sb[:])
                    last_att[0] = att
                    dst = 128 * j + 64 * h
                    nc.gpsimd.dma_start(a2a_in0[dst:dst + 64, :], att[:, 0:256])
                    nc.gpsimd.dma_start(a2a_in1[dst:dst + 64, :], att[:, 256:512])

            # ---------------- emission: interleaved schedule ----------------
            # attention(0, qt) needs q^T/k^T t-tiles <= qt and v slots
            # <= 4*qt+4, so it can start right after t-tile qt of batch 0.
            for qt in range(4):
                qkv_ttile(qt)
                v_transpose(0, range(4 * qt, 4 * qt + 4))
                attention_qtile(0, qt)
                qkv_ttile(4 + qt)
            v_transpose(1)
            for qt in range(4):
                attention_qtile(1, qt)

            # ---------------- A2A + output projection ----------------
            nc.gpsimd.collective_compute(
                "AllToAll", mybir.AluOpType.bypass,
                replica_groups=[list(range(NCORES))],
                ins=[a2a_in0[:].opt()], outs=[a2a_out0[:].opt()])
            nc.gpsimd.collective_compute(
                "AllToAll", mybir.AluOpType.bypass,
                replica_groups=[list(range(NCORES))],
                ins=[a2a_in1[:].opt()], outs=[a2a_out1[:].opt()])

            with (
                tc.tile_pool(name="attin", bufs=1) as att_in_pool,
                tc.tile_pool(name="ysb", bufs=4) as y_pool,
            ):
                # keep-warm chain pinned behind the end of attention via a
                # data dependency on the last att tile
                kw_src = att_in_pool.tile([64, 128], bf16, tag="kwsrc",
                                          name="kw_src")
                nc.vector.tensor_copy(kw_src[:], last_att[0][:, 0:128])
                junk = qkv_ps.tile([128, 512], f32, tag="qkv", name="junk")
                for _ in range(KEEPWARM):
                    nc.tensor.matmul(junk[:], kw_src[:], qT_sb[0:64, 0:512],
                                     start=True, stop=True)

                att_sbs = []
                for half, a2a_out_h in ((0, a2a_out0), (1, a2a_out1)):
                    att_sb = att_in_pool.tile([128, 8 * 256], bf16,
                                              tag=f"attfull{half}",
                                              name=f"att_sb{half}")
                    att_sbs.append(att_sb)
                    for cb in range(8):
                        nc.sync.dma_start(att_sb[:, 256 * cb:256 * cb + 256],
                                          a2a_out_h[128 * cb:128 * cb + 128, :])
                for half in range(2):
                    att_sb = att_sbs[half]
                    for qh in range(2):      # local q block within the half
                        qb = 2 * half + qh
                        for jt in range(2):
                            ps = qkv_ps.tile([128, 512], f32, tag="qkv", name="yps")
                            for cb in range(8):
                                nc.tensor.matmul(
                                    ps[:],
                                    att_sb[:, 256 * cb + 128 * qh:256 * cb + 128 * qh + 128],
                                    wo_sb[:, 1024 * cb + 512 * jt:1024 * cb + 512 * jt + 512],
                                    start=(cb == 0), stop=(cb == 7))
                            ysb = y_pool.tile([128, 512], f32, tag="ysb",
                                              name="ysb")
                            nc.vector.tensor_add(ysb[:], ps[:],
                                                 wob_sb[:, 512 * jt:512 * jt + 512])
                            nc.sync.dma_start(
                                y_out[128 * qb:128 * qb + 128, 512 * jt:512 * jt + 512],
                                ysb[:])
        ctx_lp.__exit__(None, None, None)

    nc.compile()
    return nc


def host_prep(x, Wqkv_w, Wqkv_b, Wo_w, Wo_b):
    import ml_dtypes
    bf16 = ml_dtypes.bfloat16

    x = np.asarray(x, np.float32)
    Wqkv_w = np.asarray(Wqkv_w, np.float32)
    Wqkv_b = np.asarray(Wqkv_b, np.float32)
    Wo_w = np.asarray(Wo_w, np.float32)
    Wo_b = np.asarray(Wo_b, np.float32)

    xT = np.ascontiguousarray(
        np.concatenate([x[0].T, x[1].T], axis=1), dtype=np.float32).astype(bf16)
    masks = np.zeros((128, 2048), np.float32)
    for roff in range(4):
        kk = np.arange(128)[:, None] + roff * 128
        qq = np.arange(512)[None, :]
        masks[:, 512 * roff:512 * roff + 512] = (kk <= qq).astype(np.float32)
    masks = masks.astype(bf16)
    ident = np.eye(128, dtype=np.float32).astype(bf16)
    ones64 = np.ones((128, 64), np.float32)
    onesbf = np.ones((128, 64), bf16)
    woT = np.ascontiguousarray(Wo_w.T).astype(bf16)
    wob_bcast = np.tile(Wo_b[None, :], (128, 1)).astype(np.float32)

    in_maps = []
    for i in range(NCORES):
        hA, hB = 2 * i, 2 * i + 1
        rows_qk = np.r_[64 * hA:64 * hA + 64, 64 * hB:64 * hB + 64,
                        1024 + 64 * hA:1024 + 64 * hA + 64,
                        1024 + 64 * hB:1024 + 64 * hB + 64]
        wqkT = np.ascontiguousarray(Wqkv_w[rows_qk].T).astype(bf16)
        qkb = np.ascontiguousarray(Wqkv_b[rows_qk].reshape(2, 128).T)
        rows_v = np.r_[2048 + 64 * hA:2048 + 64 * hA + 64,
                       2048 + 64 * hB:2048 + 64 * hB + 64]
        wvT = np.ascontiguousarray(Wqkv_w[rows_v].T).astype(bf16)
        vbb = np.tile(Wqkv_b[rows_v][None, :], (128, 1)).astype(np.float32)
        in_maps.append(dict(
            xT=xT, wqkT=wqkT, qk_bias=qkb, wvT=wvT, vb_bcast=vbb,
            masks=masks, ident=ident, ones64=ones64, onesbf=onesbf, woT=woT,
            wob_bcast=wob_bcast))
    return in_maps


def _ensure_ntff_hook_module():
    """run_bass_kernel_spmd(trace=True) under axon imports
    antenv.axon_hooks; provide a working ctypes-based fallback if the
    environment doesn't ship one so tracing (e.g. via BASS_TRACE=1) works."""
    import importlib
    import sys
    import types
    try:
        importlib.import_module("antenv.axon_hooks")
        return
    except ImportError:
        pass
    import contextlib
    import ctypes

    mod = types.ModuleType("antenv.axon_hooks")
    state = {"hook": None}

    def set_axon_ntff_profile_hook(h):
        state["hook"] = h

    def _make():
        try:
            lib = ctypes.CDLL("/opt/axon/libaxon_pjrt.so")
        except OSError:
            return None
        if not hasattr(lib, "axon_start_nrt_profile"):
            return None
        lib.axon_start_nrt_profile.argtypes = [
            ctypes.POINTER(ctypes.c_int64), ctypes.c_size_t]
        lib.axon_start_nrt_profile.restype = ctypes.c_int64
        lib.axon_stop_nrt_profile.argtypes = [ctypes.c_char_p]
        lib.axon_stop_nrt_profile.restype = ctypes.c_int64

        @contextlib.contextmanager
        def _hook(output_dir, device_ids):
            import jax
            jax.devices()
            if device_ids:
                ids = (ctypes.c_int64 * len(device_ids))(*device_ids)
                rc = lib.axon_start_nrt_profile(ids, len(device_ids))
            else:
                rc = lib.axon_start_nrt_profile(None, 0)
            if rc != 0:
                raise RuntimeError(f"axon_start_nrt_profile rc={rc}")
            try:
                yield
            finally:
                lib.axon_stop_nrt_profile(str(output_dir).encode())

        return _hook

    def get_axon_ntff_profile_hook():
        if state["hook"] is None:
            state["hook"] = _make()
        return state["hook"]

    mod.set_axon_ntff_profile_hook = set_axon_ntff_profile_hook
    mod.get_axon_ntff_profile_hook = get_axon_ntff_profile_hook
    try:
        import antenv
        sys.modules["antenv.axon_hooks"] = mod
        antenv.axon_hooks = mod
    except ImportError:
        pass


def kernel(x, Wqkv_w, Wqkv_b, Wo_w, Wo_b):
    from concourse import bass_utils

    _ensure_ntff_hook_module()

    if "nc" not in _cache:
        _cache["nc"] = _build()
    nc = _cache["nc"]

    in_maps = host_prep(x, Wqkv_w, Wqkv_b, Wo_w, Wo_b)
    res = bass_utils.run_bass_kernel_spmd(nc, in_maps, core_ids=list(range(NCORES)))
    _cache["last_results"] = res

    out = np.zeros((B, C, H), np.float32)
    for i in range(NCORES):
        out[i // 4, 512 * (i % 4):512 * (i % 4) + 512, :] = res.results[i]["y"]
    return out
